# revision 9
# baseline (speedup 1.0000x reference)
"""GCK 3x3 layer as a direct 3x3 valid correlation on 8 TRN2 NeuronCores.

Math: the reference's GCK basis decomposition (rowwise/colwise +-1 passes
followed by the linCombs matmul) is exactly equivalent to
    out[o, h, w] = sum_{c, dr, ds} kernels[o, c, dr, ds] * x[c, h+dr, w+ds]
with x (16, 1026, 1026), kernels (32, 16, 3, 3), out (32, 1024, 1024).

Distribution: shard output rows (height) across the 8 cores, 128 rows each;
core i gets input rows [128*i, 128*i + 130) (2-row halo), so every core is
fully local.  The tiny weight tensor is replicated.

Layouts are chosen so all DMAs are contiguous:
  x per core:  (130, 16, 1026)  row-major (host transposes the shard)
  out per core: (128, 32, 1024) h-major  (host transposes back at gather)

Per-core kernel: for each group of 4 output rows, the 6 contributing input
rows x 16 channels form a K=96 contraction (partition p = r*16 + c).  One
matmul per width tap ds (3 taps accumulated in PSUM) with stationary
W[r*16+c, ds, hrel*32+o] = kernels[o, c, r-hrel, ds];
M = 4 rows x 32 ch = 128, N = 512 (two halves of the 1024-wide row).
"""

import numpy as np

import concourse.bass as bass  # noqa: F401
import concourse.mybir as mybir
import concourse.tile as tile
from concourse import bacc
from concourse.bass_utils import run_bass_kernel_spmd

C_IN = 16
C_OUT = 32
D = 1024
W_IN = 1026
N_CORES = 8
ROWS_PER_CORE = D // N_CORES          # 128
R_IN = ROWS_PER_CORE + 2              # 130
GROUP = 4                             # output rows per matmul group
N_GROUPS = ROWS_PER_CORE // GROUP     # 32
K = C_IN * (GROUP + 2)                # 96 contraction rows

# matmul dtype: fp16 streams at 1 cycle/row (vs 4 for strict fp32), gets
# fast weight loads, and halves DMA traffic; with fp32 PSUM accumulation
# the end-to-end relative error is ~4e-4 (vs ~1.5e-4 for float32r)
MM_DT = mybir.dt.float16
NP_IN_DT = np.float16

_NC = None


def build_nc():
    nc = bacc.Bacc("TRN2", target_bir_lowering=False)
    x = nc.dram_tensor("x", [R_IN, C_IN, W_IN], MM_DT,
                       kind="ExternalInput")
    w = nc.dram_tensor("w", [K, 3, 128], MM_DT,
                       kind="ExternalInput")
    out = nc.dram_tensor("out", [ROWS_PER_CORE, C_OUT, D], MM_DT,
                         kind="ExternalOutput")

    with tile.TileContext(nc) as tc:
        with (
            tc.tile_pool(name="wpool", bufs=1) as wpool,
            tc.tile_pool(name="xpool", bufs=10) as xpool,
            tc.tile_pool(name="opool", bufs=4) as opool,
            tc.tile_pool(name="psum", bufs=6, space="PSUM") as psum,
        ):
            wt = wpool.tile([K, 3, 128], MM_DT)
            nc.sync.dma_start(wt[:], w[:])

            def do_group(xt, ot, oslot, g):
                for wh in range(2):
                    pt = psum.tile([128, 512], mybir.dt.float32)
                    for ds in range(3):
                        nc.tensor.matmul(
                            pt[:],
                            wt[:, ds, :],
                            xt[:, wh * 512 + ds: wh * 512 + ds + 512],
                            start=(ds == 0),
                            stop=(ds == 2),
                        )
                    # alternate PSUM->SBUF copies between DVE and ACT so
                    # neither engine serializes the drain
                    if wh == 0:
                        nc.vector.tensor_copy(
                            ot[:, oslot, wh * 512:(wh + 1) * 512], pt[:])
                    else:
                        nc.scalar.copy(
                            ot[:, oslot, wh * 512:(wh + 1) * 512], pt[:])

            # groups processed in pairs: per-group x loads, but one output
            # tile + one out-DMA per pair (bigger DMA batches, fewer issues)
            for gg in range(N_GROUPS // 2):
                g = 2 * gg
                ot = opool.tile([128, 2, D], MM_DT, tag="ot")
                for j in range(2):
                    xt = xpool.tile([K, W_IN], MM_DT)
                    nc.sync.dma_start(
                        xt[:],
                        x[GROUP * (g + j): GROUP * (g + j) + GROUP + 2,
                          :, :].rearrange("r c w -> (r c) w"))
                    do_group(xt, ot, j, g + j)
                nc.gpsimd.dma_start(
                    out[GROUP * g: GROUP * (g + 2), :, :].rearrange(
                        "(s hrel) o w -> (hrel o) s w", s=2),
                    ot[:])
    nc.compile()
    return nc


def prep_weights(kernels):
    """(32,16,3,3) -> stationary layout w[(hrel+dr)*16 + c, ds, hrel*32 + o]."""
    w = np.zeros((K, 3, 128), dtype=NP_IN_DT)
    for c in range(C_IN):
        for hrel in range(GROUP):
            for dr in range(3):
                # kernels[:, c, dr, :] is (o, ds); transpose to (ds, o)
                w[(hrel + dr) * 16 + c, :, hrel * 32: hrel * 32 + 32] = \
                    kernels[:, c, dr, :].T
    return w


def shard_inputs(x, kernels):
    w = prep_weights(np.asarray(kernels, dtype=np.float32))
    xf = np.asarray(x, dtype=np.float32).astype(NP_IN_DT)
    in_maps = []
    for i in range(N_CORES):
        xs = np.ascontiguousarray(
            xf[:, ROWS_PER_CORE * i: ROWS_PER_CORE * i + R_IN, :]
            .transpose(1, 0, 2))
        in_maps.append({"x": xs, "w": w})
    return in_maps


def gather(results):
    # per-core out is (128, 32, 1024) h-major; stitch rows then go o-major
    full = np.concatenate([r["out"] for r in results], axis=0)
    return np.ascontiguousarray(full.transpose(1, 0, 2).astype(np.float32))


def kernel(x, kernels):
    global _NC
    if _NC is None:
        _NC = build_nc()
    in_maps = shard_inputs(x, kernels)
    res = run_bass_kernel_spmd(_NC, in_maps, core_ids=list(range(N_CORES)))
    return gather(res.results)
